# revision 60
# baseline (speedup 1.0000x reference)
"""Trainium2 Bass kernel for nn_CycleGNN (8-step projected-direction solver).

Contract: kernel(**inputs) takes the FULL unsharded numpy inputs (keyed as in
setup_inputs()) and returns the full output (preds, labels), each
[131072, 8] float32.  Internally shards the 64 graphs across 8 NeuronCores
(8 graphs per core, graphs never interact -> no collectives), runs a Tile
kernel via run_bass_kernel_spmd, and re-assembles on the host.

Device-side layout notes (per core, 8 graphs, 16384 nodes):
 - per-node state is "p-major banded" [128, 128]: tile[p, c] = v[p*128 + c];
   graph g owns partitions [16g, 16g+16).
 - BOTH P and P^T are SBUF-resident in fp8-e3m4 (scaled by 32; 64KB/partition
   each), so the steady-state loop does no HBM traffic at all.
 - einsum1 (df = P^T d) and einsum2 (y = P df) run as 4-way column-tiled
   matvecs (4 graphs concurrent in the PE array via tile_position), with
   bf16 moving d/df columns against the fp8 resident operand.
 - PSUM matvec rows are evacuated with full-tile [128, 512] ops (rows
   0/32/64/96 hold real data) instead of per-row [1, 512] single-lane
   copies, then re-banded by partition-scatter DMAs.
"""

import numpy as np
import ml_dtypes

import bass_rust
import concourse.bass as bass
import concourse.tile as tile
from concourse import mybir
from concourse.bass_utils import run_bass_kernel_spmd
from concourse.masks import make_identity

F32 = mybir.dt.float32
BF16 = mybir.dt.bfloat16
FP8 = mybir.dt.float8e3
FP8E4 = mybir.dt.float8e4
BF = ml_dtypes.bfloat16
F8 = ml_dtypes.float8_e3m4
F8E4 = ml_dtypes.float8_e4m3
PSCALE = 32.0    # P^T (einsum2) stored as fp8e3 * PSCALE (absmax ~3.8 < 15.5)
E1_DR = False    # einsum1 via DoubleRow fp8e4: measured slower on HW + tighter
                 # labels margin (1.69e-2 vs 1.01e-2) -> keep fp8e3 at 1x
P1SCALE = 16.0   # P (einsum1) stored as fp8e4 * P1SCALE when E1_DR
DSCALE = 64.0    # d stored as fp8e4 * DSCALE when E1_DR (|d|<=3 -> <=192 < 240)

B = 64          # graphs
NMAX = 2048     # nodes per graph (equal-size, sorted vals_batch)
F = 512         # projection basis dim
HID = 128
NFEAT = 64
NUM_STEPS = 8
STEP_ALPHA = 5.0
NCORES = 8
GPC = B // NCORES            # graphs per core = 8
NPC = GPC * NMAX             # nodes per core = 16384
NCH = NMAX // 128            # n-chunks per graph = 16
FCH = F // 128               # f-chunks = 4
NODE_CH = NPC // 512         # mlp node chunks of 512 = 32

AX = mybir.AxisListType
OP = mybir.AluOpType
ACT = mybir.ActivationFunctionType

_COMPILED = {}


def _split_sync_waits(nc, maxw=1):
    """Walrus in this container accepts at most one sync wait per
    instruction; split extra waits into preceding engine-local NoOps."""
    ctr = 0
    for f in nc.m.functions:
        for bb in f.blocks:
            insts = bb.instructions
            out = []
            changed = False
            for ins in insts:
                si = ins.sync_info
                waits = list(si.on_wait) if si is not None else []
                if len(waits) > maxw:
                    reg_waits = [w for w in waits if w.wait_reg is not None]
                    imm_waits = [w for w in waits if w.wait_reg is None]
                    nkeep = max(0, maxw - len(reg_waits))
                    keep = imm_waits[:nkeep]
                    extra = imm_waits[nkeep:]
                    for i in range(0, len(extra), maxw):
                        ctr += 1
                        nop = mybir.InstNoOp(name=f"wsplit-{ctr}", ins=[], outs=[])
                        nop.engine = ins.engine
                        nop.sync_info = bass_rust.SyncInfo(
                            on_wait=extra[i : i + maxw], on_update=[]
                        )
                        out.append(nop)
                    ins.sync_info = bass_rust.SyncInfo(
                        on_wait=reg_waits + keep, on_update=list(si.on_update)
                    )
                    changed = True
                out.append(ins)
            if changed:
                bb.instructions = out
    return ctr


def _tau_schedule():
    taus = []
    tau = 0.01
    for _ in range(NUM_STEPS):
        taus.append(tau)
        tau = max(tau * 0.5, 1e-5)
    return taus


def build_nc(num_steps=NUM_STEPS, skip=()):
    nc = bass.Bass()

    # ---------------- I/O ----------------
    P_d = nc.declare_dram_parameter(
        "P", [128, GPC, NCH, F], FP8E4 if E1_DR else FP8, isOutput=False
    )
    PT_d = nc.declare_dram_parameter("PT", [128, GPC, FCH, NMAX], FP8, isOutput=False)
    nfT_d = nc.declare_dram_parameter("nfT", [NFEAT, NPC], BF16, isOutput=False)
    xs0_d = nc.declare_dram_parameter("xs0", [128, 128], F32, isOutput=False)
    xsol_d = nc.declare_dram_parameter("xsol", [128, 128], F32, isOutput=False)
    w1_d = nc.declare_dram_parameter("w1", [NFEAT + 1, HID], BF16, isOutput=False)
    b1_d = nc.declare_dram_parameter("b1", [HID, 1], F32, isOutput=False)
    w2_d = nc.declare_dram_parameter("w2", [HID, 1], BF16, isOutput=False)
    b2_d = nc.declare_dram_parameter("b2", [1, 1], F32, isOutput=False)
    seg_d = nc.declare_dram_parameter("seg", [128, 128], F32, isOutput=False)
    seg8_d = nc.declare_dram_parameter("seg8", [GPC, 128], F32, isOutput=False)

    preds_o = nc.declare_dram_parameter("preds", [NUM_STEPS, NPC], F32, isOutput=True)
    # xs snapshot at the START of each step; labels are computed on the host
    xs_o = nc.declare_dram_parameter("xs_o", [NUM_STEPS, 128, 128], F32, isOutput=True)

    taus = _tau_schedule()

    with tile.TileContext(nc) as tc:
        with (
            tc.tile_pool(name="res", bufs=1) as res,            # resident singles
            tc.tile_pool(name="hp", bufs=6) as hp,              # relu'd hidden chunks
            tc.tile_pool(name="rows", bufs=4) as rows,          # row staging
            tc.tile_pool(name="smt", bufs=1) as smt,            # small temps / state
            tc.tile_pool(name="mlp_p_ps", bufs=3, space="PSUM") as mlp_p_ps,
            tc.tile_pool(name="ei_ps", bufs=1, space="PSUM") as ei_ps,
            tc.tile_pool(name="ms_ps", bufs=1, space="PSUM") as ms_ps,
        ):
            # ---------------- constants / residents ----------------
            identf = res.tile([128, 128], F32, tag="identf")
            make_identity(nc, identf)
            identb = res.tile([128, 128], BF16, tag="identb")
            make_identity(nc, identb)

            seg = res.tile([128, 128], F32, tag="seg")
            nc.sync.dma_start(out=seg, in_=seg_d[:])
            seg8 = res.tile([GPC, 128], F32, tag="seg8")
            nc.sync.dma_start(out=seg8, in_=seg8_d[:])

            w1 = res.tile([NFEAT + 1, HID], BF16, tag="w1")
            nc.sync.dma_start(out=w1, in_=w1_d[:])
            b1c = res.tile([HID, 1], F32, tag="b1c")
            nc.sync.dma_start(out=b1c, in_=b1_d[:])
            w2 = res.tile([HID, 1], BF16, tag="w2")
            nc.sync.dma_start(out=w2, in_=w2_d[:])
            b2c = res.tile([128, 1], F32, tag="b2c")
            nc.sync.dma_start(
                out=b2c,
                in_=bass.AP(tensor=b2_d, offset=0, ap=[[0, 128], [1, 1]]),
            )

            # mlp moving operand: rows 0..63 node features, row 64 = xs
            rhsx = res.tile([NFEAT + 1, NPC], BF16, tag="rhsx")
            nc.sync.dma_start(out=rhsx[0:NFEAT, :], in_=nfT_d[:])

            # state
            xs = res.tile([128, 128], F32, tag="xs")
            nc.sync.dma_start(out=xs, in_=xs0_d[:])
            xsol = res.tile([128, 128], F32, tag="xsol")
            nc.sync.dma_start(out=xsol, in_=xsol_d[:])

            # resident P and P^T (fp8 * scale, 64KB/partition each), queued
            # LAST so step 0's MLP overlaps the ~90us load; e1 needs sbP
            # first, so P streams before P^T.
            sbP = res.tile([128, GPC, NCH, F], FP8E4 if E1_DR else FP8, tag="sbP")
            nc.sync.dma_start(out=sbP, in_=P_d[:])
            sbPT = res.tile([128, GPC, FCH, NMAX], FP8, tag="sbPT")
            nc.sync.dma_start(out=sbPT, in_=PT_d[:])

            for s in range(num_steps):
                tau = taus[s]

                # ---- A: xs row (bf16) into rhsx[64]; snapshot xs for host ----
                xs_bf = smt.tile([128, 128], BF16, tag="xs_bf")
                nc.vector.tensor_copy(xs_bf, xs)
                nc.scalar.dma_start(
                    out=rhsx[NFEAT : NFEAT + 1, :].rearrange(
                        "o (p c) -> o p c", p=128
                    ),
                    in_=xs_bf,
                )
                nc.gpsimd.dma_start(out=xs_o[s], in_=xs)
                # rterm = 3*tau / (xs + tau), needed only after the MLP
                rterm = smt.tile([128, 128], F32, tag="rterm")
                nc.vector.tensor_scalar(
                    out=rterm, in0=xs, scalar1=float(tau),
                    scalar2=float(1.0 / (3.0 * tau)), op0=OP.add, op1=OP.mult,
                )
                nc.vector.reciprocal(rterm, rterm)

                # ---- B+C: MLP ----
                pred = smt.tile([128, 128], BF16, tag="pred", bufs=2)
                prow = rows.tile([128, 4096], BF16, tag="prow", bufs=2)
                for r in range(NODE_CH // 4):  # 8 rounds, groups interleaved
                    pp = mlp_p_ps.tile([128, 512], F32, tag="predp")
                    for q in range(4):
                        j = 8 * q + r  # chunk -> col group q = j // 8
                        hpsum = ei_ps.tile([128, 512], F32, tag=f"yp{q}", name="hpsum")
                        NWM = 64 if "mlpmm" in skip else 512
                        nc.tensor.matmul(
                            hpsum[:, 0:NWM],
                            w1,
                            rhsx[:, 512 * j : 512 * j + NWM],
                            start=True,
                            stop=True,
                        )
                        hpos = hp.tile([128, 512], BF16, tag="hpos")
                        NWR = 64 if "relu" in skip else 512
                        if q % 2 == 0:
                            nc.vector.tensor_scalar(
                                out=hpos[:, 0:NWR], in0=hpsum[:, 0:NWR],
                                scalar1=b1c, scalar2=0.0,
                                op0=OP.add, op1=OP.max,
                            )
                        else:
                            nc.scalar.activation(
                                out=hpos[:, 0:NWR], in_=hpsum[:, 0:NWR],
                                func=ACT.Relu, bias=b1c,
                            )
                        if NWR < 512:
                            nc.vector.tensor_copy(hpos[:, NWR:512], hpos[:, 0:512-NWR])
                        nc.tensor.matmul(
                            pp[32 * q : 32 * q + 1, :],
                            w2,
                            hpos,
                            start=True,
                            stop=True,
                            tile_position=(0, 32 * q),
                        )
                    # ganged evac of the 4 pred rows (+b2): one [128, 512] op
                    NEp = 64 if "evac" in skip else 512
                    if r % 2 == 0:
                        nc.vector.tensor_scalar(
                            out=prow[:, 512 * r : 512 * r + NEp],
                            in0=pp[:, 0:NEp],
                            scalar1=b2c, scalar2=None, op0=OP.add,
                        )
                    else:
                        nc.scalar.activation(
                            out=prow[:, 512 * r : 512 * r + NEp],
                            in_=pp[:, 0:NEp],
                            func=ACT.Identity, bias=b2c,
                        )
                # 4 batched scatters: group q holds nodes [4096q, 4096(q+1))
                for q in range(4):
                    eng = (nc.sync, nc.scalar, nc.gpsimd, nc.sync)[q]
                    eng.dma_start(
                        out=pred[32 * q : 32 * q + 32, :],
                        in_=prow[32 * q : 32 * q + 1, :].rearrange(
                            "o (p c) -> o p c", p=32
                        ),
                    )
                # preds output straight from the row staging (flat node order)
                nc.gpsimd.dma_start(
                    out=preds_o[s].rearrange("(q c) -> q c", q=4),
                    in_=prow.rearrange("(q o) c -> q o c", q=4)[:, 0:1, :],
                )

                # d operand for einsum1 DoubleRow: block-diagonal graph pairs
                # (col 0 = (d_A, 0), col 1 = (0, d_B)) so one instruction
                # computes both graphs' df at 2 fp8/lane/cycle. Ko needs a
                # 16-byte stride, hence the padded last dim; off-diagonal
                # zeros are memset once at kernel start.
                if E1_DR:
                    d_bd = smt.tile([128, GPC // 2, NCH, 2, 16], FP8E4, tag="d_bd")
                    if s == 0:
                        nc.vector.memset(d_bd, 0.0)
                else:
                    d_c = smt.tile([128, 128], BF16, tag="d_c")
                do_d = "dchain" not in skip
                if not do_d:
                    if E1_DR:
                        nc.vector.memset(d_bd, 0.01)
                    else:
                        nc.vector.memset(d_c, 0.01)
                if do_d:
                    # ---- D+E: direction d = pred/||pred||_1 + 3tau/(xs+tau) ----
                    psum_part = smt.tile([128, 1], F32, tag="psum_part")
                    junk = smt.tile([128, 128], F32, tag="junk", bufs=1)
                    nc.scalar.activation(
                        out=junk, in_=pred, func=ACT.Abs, accum_out=psum_part
                    )
                    gs_ps = ms_ps.tile([128, 1], F32, tag="ms")
                    nc.tensor.matmul(gs_ps, seg, psum_part, start=True, stop=True)
                    pscale = smt.tile([128, 1], F32, tag="pscale")
                    nc.vector.reciprocal(pscale, gs_ps)
                    d_bf = smt.tile([128, 128], BF16, tag="d_bf")
                    nc.vector.scalar_tensor_tensor(
                        out=d_bf, in0=pred, scalar=pscale, in1=rterm,
                        op0=OP.mult, op1=OP.add,
                    )
                    dct_ps = ms_ps.tile([128, 128], BF16, tag="ms")
                    nc.tensor.transpose(dct_ps, d_bf, identb)
                    if E1_DR:
                        for gg in range(2):
                            nc.vector.tensor_scalar(
                                out=d_bd[:, :, :, gg, gg : gg + 1],
                                in0=dct_ps.rearrange(
                                    "p (pr two k x) -> p pr two k x",
                                    pr=GPC // 2, two=2, x=1,
                                )[:, :, gg, :, :],
                                scalar1=DSCALE, scalar2=None, op0=OP.mult,
                            )
                    else:
                        nc.vector.tensor_copy(d_c, dct_ps)

                # ---- F: einsum1  df[g,f] = sum_n P[g,n,f] d[g,n] ----
                df_cols = smt.tile([128, GPC, FCH], BF16, tag="df_cols")
                if "dfprep" in skip:
                    nc.vector.memset(df_cols, 0.01)
                NW1 = 64 if "e1mm" in skip else 512
                e1scale = 1.0 / (P1SCALE * DSCALE) if E1_DR else 1.0 / PSCALE
                if E1_DR:
                    # 4 graph-pairs, one PSUM bank each, df rows at {0, 1}
                    dfps = [
                        ei_ps.tile([128, F], F32, tag=f"yp{pr}", name="dfp")
                        for pr in range(4)
                    ]
                    for k in range(NCH):
                        for pr in range(4):
                            nc.tensor.matmul(
                                dfps[pr][0:2, 0:NW1],
                                d_bd[:, pr, k, :, 0:2],
                                sbP[:, 2 * pr : 2 * pr + 2, k, 0:NW1],
                                start=(k == 0),
                                stop=(k == NCH - 1),
                                perf_mode=mybir.MatmulPerfMode.DoubleRow,
                            )
                    if "dfprep" not in skip:
                        for pr in range(4):
                            dfstage = rows.tile(
                                [128, F], BF16, tag=f"dfstage{pr % 2}", bufs=2
                            )
                            if pr % 2 == 0:
                                nc.scalar.activation(
                                    out=dfstage, in_=dfps[pr],
                                    func=ACT.Identity, scale=e1scale,
                                )
                            else:
                                nc.vector.tensor_scalar(
                                    out=dfstage, in0=dfps[pr],
                                    scalar1=e1scale, scalar2=None, op0=OP.mult,
                                )
                            for k in range(FCH):
                                tp_ps = ei_ps.tile(
                                    [128, 128], BF16, tag=f"yp{pr}", name="tp"
                                )
                                nc.tensor.transpose(
                                    tp_ps, dfstage[:, 128 * k : 128 * (k + 1)], identb
                                )
                                # columns {0, 1} hold the pair's df chunks
                                nc.vector.tensor_copy(
                                    df_cols[:, 2 * pr : 2 * pr + 2, k : k + 1],
                                    tp_ps.rearrange("p (a b) -> p a b", b=1)[
                                        :, 0:2, 0:1
                                    ],
                                )
                else:
                  for grp in range(2):
                    dfp = ei_ps.tile([128, F], F32, tag=f"yp{grp}", name="dfp")
                    for k in range(NCH):
                        for g4 in range(4):
                            g = 4 * grp + g4
                            nc.tensor.matmul(
                                dfp[32 * g4 : 32 * g4 + 1, 0:NW1],
                                d_c[:, 16 * g + k : 16 * g + k + 1],
                                sbP[:, g, k, 0:NW1],
                                start=(k == 0),
                                stop=(k == NCH - 1),
                                tile_position=(0, 32 * g4),
                            )
                    if "dfprep" not in skip:
                        # ganged evac (descale) -> bf16 staging, 4 transposes
                        dfstage = rows.tile([128, F], BF16, tag="dfstage", bufs=2)
                        nc.scalar.activation(
                            out=dfstage, in_=dfp, func=ACT.Identity, scale=e1scale,
                        )
                        for k in range(FCH):
                            tp_ps = ei_ps.tile(
                                [128, 128], BF16, tag=f"yp{2 + k % 2}", name="tp"
                            )
                            nc.tensor.transpose(
                                tp_ps, dfstage[:, 128 * k : 128 * (k + 1)], identb
                            )
                            # columns {0,32,64,96}: df chunks of the 4 graphs
                            nc.vector.tensor_copy(
                                df_cols[:, 4 * grp : 4 * grp + 4, k : k + 1],
                                tp_ps.rearrange("p (a b) -> p a b", b=32)[:, :, 0:1],
                            )

                # ---- G: einsum2, with the per-half line-search DVE chain
                # emitted after each half's scatters so half 0's chain hides
                # under half 1's matmuls ----
                y_pm = smt.tile([128, 128], BF16, tag="y_pm", bufs=2)
                q = smt.tile([128, 128], F32, tag="q", bufs=1)
                stp = smt.tile([128, 128], F32, tag="stp", bufs=1)
                smin = smt.tile([128, 1], F32, tag="smin")
                for grp in range(2):
                    yps = [ei_ps.tile([128, 512], F32, tag=f"yp{j}", name=f"yp{j}") for j in range(4)]
                    yrow = rows.tile([128, 2048], BF16, tag="yrow", bufs=2)
                    for k in range(FCH):
                        NW = 64 if "e2mm" in skip else 512
                        for j in range(4):
                            for g4 in range(4):
                                g = 4 * grp + g4
                                nc.tensor.matmul(
                                    yps[j][32 * g4 : 32 * g4 + 1, 0:NW],
                                    df_cols[:, g, k : k + 1],
                                    sbPT[:, g, k, 512 * j : 512 * j + NW],
                                    start=(k == 0),
                                    stop=(k == FCH - 1),
                                    tile_position=(0, 32 * g4),
                                )
                    # ganged evac (descale): one [128, 512] op per j-chunk
                    NE = 64 if "evac" in skip else 512
                    for j in range(4):
                        if j % 2 == 0:
                            nc.vector.tensor_scalar(
                                out=yrow[:, 512 * j : 512 * j + NE],
                                in0=yps[j][:, 0:NE],
                                scalar1=1.0 / PSCALE, scalar2=None, op0=OP.mult,
                            )
                        else:
                            nc.scalar.activation(
                                out=yrow[:, 512 * j : 512 * j + NE],
                                in_=yps[j][:, 0:NE],
                                func=ACT.Identity, scale=1.0 / PSCALE,
                            )
                    for g4 in range(4):
                        g = 4 * grp + g4
                        eng = (nc.sync, nc.scalar, nc.gpsimd, nc.sync)[g4]
                        eng.dma_start(
                            out=y_pm[16 * g : 16 * g + 16, :],
                            in_=yrow[32 * g4 : 32 * g4 + 1, :].rearrange(
                                "o (p c) -> o p c", p=16
                            ),
                        )
                    if "alpha" not in skip:
                        # per-half line-search DVE chain
                        sl = slice(64 * grp, 64 * grp + 64)
                        nc.vector.tensor_scalar(
                            out=q[sl, :], in0=y_pm[sl, :], scalar1=-1.0,
                            scalar2=1e-30, op0=OP.mult, op1=OP.max,
                        )
                        nc.vector.reciprocal(q[sl, :], q[sl, :])
                        nc.vector.tensor_mul(stp[sl, :], xs[sl, :], q[sl, :])
                        nc.vector.tensor_reduce(
                            out=smin[sl, :], in_=stp[sl, :], axis=AX.X, op=OP.min
                        )

                # ---- H: line search tail + state update ----
                if "alpha" in skip:
                    nc.vector.scalar_tensor_tensor(
                        out=xs, in0=y_pm, scalar=0.05, in1=xs,
                        op0=OP.mult, op1=OP.add,
                    )
                    continue
                # per-graph min: transpose -> row -> seg-min -> replicate back
                smin_ps = ms_ps.tile([128, 128], F32, tag="ms")
                nc.tensor.transpose(smin_ps[0:1, :], smin, identf)
                amin_row = smt.tile([1, GPC], F32, tag="amin_row")
                nc.vector.tensor_reduce(
                    out=amin_row,
                    in_=smin_ps[0:1, :].rearrange("o (g b) -> o g b", g=GPC),
                    axis=AX.X,
                    op=OP.min,
                )
                nc.vector.tensor_scalar(
                    out=amin_row, in0=amin_row, scalar1=float(STEP_ALPHA),
                    scalar2=0.995, op0=OP.min, op1=OP.mult,
                )
                # replicate per-graph alpha to its 16-partition band
                a8_ps = ms_ps.tile([GPC, 1], F32, tag="ms")
                nc.tensor.transpose(a8_ps, amin_row, identf[0:1, 0:1])
                a8 = smt.tile([GPC, 1], F32, tag="a8")
                nc.vector.tensor_copy(a8, a8_ps)
                acol_ps = ms_ps.tile([128, 1], F32, tag="ms")
                nc.tensor.matmul(acol_ps, seg8, a8, start=True, stop=True)
                acol = smt.tile([128, 1], F32, tag="acol")
                nc.vector.tensor_copy(acol, acol_ps)
                nc.vector.scalar_tensor_tensor(
                    out=xs, in0=y_pm, scalar=acol, in1=xs,
                    op0=OP.mult, op1=OP.add,
                )

    _split_sync_waits(nc, maxw=1)
    return nc


def _seg_mats():
    seg = np.zeros((128, 128), np.float32)
    for g in range(GPC):
        seg[16 * g : 16 * g + 16, 16 * g : 16 * g + 16] = 1.0
    seg8 = np.zeros((GPC, 128), np.float32)
    for g in range(GPC):
        seg8[g, 16 * g : 16 * g + 16] = 1.0
    return seg, seg8


def _prep_core_inputs(core, proj, x_start, x_solution, node_feat, W1, b1, W2, b2):
    g0 = core * GPC
    n0 = core * NPC
    Pc = proj[g0 : g0 + GPC]  # [8, 2048, 512] f32
    P_f8 = np.ascontiguousarray(
        Pc.reshape(GPC, NCH, 128, F).transpose(2, 0, 1, 3)
        * (P1SCALE if E1_DR else PSCALE)
    ).astype(F8E4 if E1_DR else F8)
    PT_f8 = np.ascontiguousarray(
        (Pc * PSCALE).transpose(0, 2, 1).reshape(GPC, FCH, 128, NMAX)
        .transpose(2, 0, 1, 3)
    ).astype(F8)
    nfT = np.ascontiguousarray(node_feat[n0 : n0 + NPC].T).astype(BF)
    seg, seg8 = _seg_mats()
    return {
        "P": P_f8,
        "PT": PT_f8,
        "nfT": nfT,
        "xs0": x_start[n0 : n0 + NPC].reshape(128, 128).astype(np.float32),
        "xsol": x_solution[n0 : n0 + NPC].reshape(128, 128).astype(np.float32),
        "w1": W1.astype(BF),
        "b1": b1.reshape(HID, 1).astype(np.float32),
        "w2": W2.reshape(HID, 1).astype(BF),
        "b2": b2.reshape(1, 1).astype(np.float32),
        "seg": seg,
        "seg8": seg8,
    }


def _numpy_fallback(x_start, x_solution, node_feat, proj_matrix, W1, b1, W2, b2, batch):
    """General (ragged) reference implementation in numpy, used only if
    vals_batch is not the expected equal-size pattern."""
    nb = proj_matrix.shape[0]
    batch = batch.astype(np.int64)
    counts = np.bincount(batch, minlength=nb)
    offsets = np.cumsum(counts) - counts
    pos = np.arange(batch.shape[0]) - offsets[batch]

    def l1norm(x):
        s = np.zeros(nb, x.dtype)
        np.add.at(s, batch, np.abs(x))
        return x / np.clip(s, 1e-8, None)[batch]

    def to_dense(x):
        dense = np.zeros((nb, NMAX), x.dtype)
        m = pos < NMAX
        dense[batch[m], pos[m]] = x[m]
        return dense

    def line_search(x, dvec):
        neg = dvec < 0
        step = np.where(neg, x / np.where(neg, -dvec, 1.0), STEP_ALPHA)
        a = np.full(nb, np.inf, step.dtype)
        np.minimum.at(a, batch, step)
        return np.minimum(a, STEP_ALPHA)[batch]

    def gnn(x):
        h = np.concatenate([node_feat, x[:, None]], axis=-1)
        h = np.maximum(h @ W1 + b1, 0.0)
        return (h @ W2 + b2)[:, 0]

    tau = 0.01
    xs = x_start.astype(np.float32)
    preds, labels = [], []
    for _ in range(NUM_STEPS):
        pred = gnn(xs)
        preds.append(pred)
        labels.append(l1norm(x_solution - xs))
        p = l1norm(pred)
        direction = p + 3.0 * tau / (xs + tau)
        tau = max(tau * 0.5, 1e-5)
        d_dense = to_dense(direction)
        df = np.einsum("bnf,bn->bf", proj_matrix, d_dense)
        proj_dense = np.einsum("bnf,bf->bn", proj_matrix, df)
        proj_flat = proj_dense[batch, np.minimum(pos, NMAX - 1)]
        proj_flat = np.where(pos < NMAX, proj_flat, 0.0)
        alpha = line_search(xs, proj_flat) * 0.995
        xs = xs + alpha * proj_flat
    return np.stack(preds, 1).astype(np.float32), np.stack(labels, 1).astype(np.float32)


def run_on_hw(inputs_list):
    if "plain" not in _COMPILED:
        _COMPILED["plain"] = build_nc()
    nc = _COMPILED["plain"]
    return run_bass_kernel_spmd(nc, inputs_list, list(range(NCORES))).results


def kernel(x_start, x_solution, node_feat, proj_matrix, W1, b1, W2, b2, vals_batch):
    expected = np.repeat(np.arange(B, dtype=np.int64), NMAX)
    vb = np.asarray(vals_batch)
    if vb.shape != expected.shape or not np.array_equal(
        vb.astype(np.int64), expected
    ):
        return _numpy_fallback(
            np.asarray(x_start, np.float32),
            np.asarray(x_solution, np.float32),
            np.asarray(node_feat, np.float32),
            np.asarray(proj_matrix, np.float32),
            np.asarray(W1, np.float32),
            np.asarray(b1, np.float32),
            np.asarray(W2, np.float32),
            np.asarray(b2, np.float32),
            vb,
        )

    x_start = np.asarray(x_start, np.float32)
    x_solution = np.asarray(x_solution, np.float32)
    node_feat = np.asarray(node_feat, np.float32)
    proj_matrix = np.asarray(proj_matrix, np.float32)
    W1 = np.asarray(W1, np.float32)
    b1 = np.asarray(b1, np.float32)
    W2 = np.asarray(W2, np.float32)
    b2 = np.asarray(b2, np.float32)

    ins = [
        _prep_core_inputs(c, proj_matrix, x_start, x_solution, node_feat, W1, b1, W2, b2)
        for c in range(NCORES)
    ]
    results = run_on_hw(ins)
    preds = np.concatenate(
        [results[c]["preds"].T for c in range(NCORES)], axis=0
    ).astype(np.float32)
    # labels = l1norm(x_solution - xs_s) from the per-step xs snapshots
    xs_all = np.concatenate(
        [results[c]["xs_o"].reshape(NUM_STEPS, NPC) for c in range(NCORES)], axis=1
    )  # [NUM_STEPS, TOTAL]
    diff = x_solution[None, :] - xs_all
    d3 = diff.reshape(NUM_STEPS, B, NMAX)
    sums = np.clip(np.abs(d3).sum(axis=2, keepdims=True), 1e-8, None)
    labels = np.ascontiguousarray(
        (d3 / sums).reshape(NUM_STEPS, B * NMAX).T
    ).astype(np.float32)
    return preds, labels
